# revision 1
# baseline (speedup 1.0000x reference)
"""Trainium2 Bass/Tile kernel for the gnn_message_passing problem.

Math (per batch element b, with x = ftr[b] viewed as [C, HW]):
    avg[c] = mean_n x[c,n];  mx[c] = max_n x[c,n]
    cw     = sigmoid(relu(Wa @ avg) + relu(Wm @ mx))              [M]
    k      = relu(Wk @ x + bk)                                    [M, HW]
    kq     = cw[:,None] * k
    S      = sigmoid(kq^T k)   (symmetric!)                       [HW, HW]
    d      = (S @ 1)^(-1/2)                                       [HW]
    kdT    = d[:,None] * k^T ;  kdc = cw[:,None] * d[None,:] * k
    A      = (d*k) @ x^T  (contract HW)                           [M, C]
    AG     = A @ gcn_w                                            [M, C]
    out    = (I + gcn_w)^T @ x - sum_m AG[m,c] kdc[m,n]           [C, HW]

Fused-tail derivation (no LX / mid intermediates):
    mid = cw (.) A;  LX^T = x - sum_m mid[m,c] (d*k)[m,n];  Y^T = x + g^T LX^T
    Y^T = x + g^T x - g^T T  with  T[c,n] = sum_m mid[m,c] (d*k)[m,n]
    g^T T = sum_m (A @ g)[m,c] * (cw*d*k)[m,n] = sum_m AG[m,c] kdc[m,n]
    x + g^T x = (I + g)^T x  -> single "gplus" stationary operand.

S is symmetric, so only the upper triangle of S (by 128-row strips) is
computed: row-sums via the scalar engine's fused sigmoid+accumulate; the
missing lower-triangle contributions are column-sums of each strip, computed
on the PE as ones-column matmuls into a persistent PSUM accumulator G
([strip, col] layout), finally reduced with per-tile ones-matvecs.

Sharding: data-parallel over batch B=8 across 8 cores (1 image per core),
weights replicated. No collectives.
"""

import numpy as np
from contextlib import ExitStack

import concourse.bass as bass
import concourse.mybir as mybir
import concourse.tile as tile
from concourse import bacc
from concourse.bass_utils import run_bass_kernel_spmd
from concourse.masks import make_identity

F32 = mybir.dt.float32
F32R = mybir.dt.float32r  # fp32 bits, reduced-precision 4x-faster PE mode
BF16 = mybir.dt.bfloat16
AF = mybir.ActivationFunctionType
AX = mybir.AxisListType

B, C, H, W = 8, 256, 48, 48
HW = H * W            # 2304
M = 128               # C // 2
P = 128               # partitions
CT = C // P           # 2 c-tiles
NT = HW // P          # 18 n-tiles
N_CORES = 8
BANK = 512            # fp32 elements per PSUM bank


def _chunks(total, step, start=0):
    out = []
    off = start
    while off < total:
        sz = min(step, total - off)
        out.append((off, sz))
        off += sz
    return out


def _bank_chunks(start, end):
    """[start, end) split at PSUM bank boundaries."""
    out = []
    off = start
    while off < end:
        nxt = min(end, (off // BANK + 1) * BANK)
        out.append((off, nxt - off))
        off = nxt
    return out


TRIANGLE = True


def build_program(reps=1, triangle=None):
    global TRIANGLE
    if triangle is not None:
        TRIANGLE = triangle
    nc = bacc.Bacc("TRN2", target_bir_lowering=False, debug=False)

    ftr = nc.declare_dram_parameter("ftr", [C, HW], F32, isOutput=False)
    convw = nc.declare_dram_parameter("convw", [M, C], F32, isOutput=False)
    convb = nc.declare_dram_parameter("convb", [M, 1], F32, isOutput=False)
    avgw = nc.declare_dram_parameter("avgw", [M, C], F32, isOutput=False)
    maxw = nc.declare_dram_parameter("maxw", [M, C], F32, isOutput=False)
    gcnw = nc.declare_dram_parameter("gcnw", [C, C], F32, isOutput=False)
    out = nc.declare_dram_parameter("out", [C, HW], F32, isOutput=True)

    with tile.TileContext(nc) as tc:
        for _ in range(reps):
            with ExitStack() as ctx:
                _body(ctx, tc, ftr, convw, convb, avgw, maxw, gcnw, out)
    nc.compile()
    return nc


def _body(ctx, tc, ftr, convw, convb, avgw, maxw, gcnw, out):
    nc = tc.nc

    sb = ctx.enter_context(tc.tile_pool(name="sb", bufs=1))
    scr = ctx.enter_context(tc.tile_pool(name="scr", bufs=4))
    mmp = ctx.enter_context(tc.tile_pool(name="mmp", bufs=1, space="PSUM"))

    # ---- persistent SBUF tiles ----
    x_sb = sb.tile([P, CT, HW], F32, tag="x")
    xr_sb = sb.tile([P, CT, HW], F32R, tag="xr")
    xT_sb = sb.tile([P, NT, C], F32R, tag="xT")
    k_sb = sb.tile([P, HW], F32R, tag="k")
    kq_sb = sb.tile([P, HW], F32R, tag="kq")
    kT_sb = sb.tile([P, NT, M], F32R, tag="kT")
    kdT_sb = sb.tile([P, NT, M], F32R, tag="kdT")
    kd_sb = sb.tile([P, HW], F32R, tag="kd")      # holds cw*d*k (kdc)
    yT_sb = sb.tile([P, CT, HW], F32, tag="yT")
    G_sb = sb.tile([P, HW], F32, tag="G")         # strip colsums [strip, col]

    convw_sb = sb.tile([P, C], F32, tag="convw")
    convwT_sb = sb.tile([P, CT, M], F32R, tag="convwT")
    convb_sb = sb.tile([P, 1], F32, tag="convb")
    avgw_sb = sb.tile([P, C], F32, tag="avgw")
    avgwT_sb = sb.tile([P, CT, M], F32, tag="avgwT")
    maxw_sb = sb.tile([P, C], F32, tag="maxw")
    maxwT_sb = sb.tile([P, CT, M], F32, tag="maxwT")
    g_sb = sb.tile([P, CT, C], F32, tag="g")
    gr_sb = sb.tile([P, CT, C], F32R, tag="gr")         # rounded gcn_w
    gplus_sb = sb.tile([P, CT, C], F32R, tag="gplus")   # gcn_w + I, rounded
    ident = sb.tile([P, P], F32, tag="ident")
    identr = sb.tile([P, P], F32R, tag="identr")
    q_sb = sb.tile([P, 2 * P], BF16, tag="q")     # ones-column selector
    ones_sb = sb.tile([P, 1], F32, tag="ones")

    avg_sb = sb.tile([P, CT, 1], F32, tag="avg")
    mx_sb = sb.tile([P, CT, 1], F32, tag="mx")
    ra_sb = sb.tile([P, 1], F32, tag="ra")
    rm_sb = sb.tile([P, 1], F32, tag="rm")
    cwin_sb = sb.tile([P, 1], F32, tag="cwin")
    cw_sb = sb.tile([P, 1], F32, tag="cw")
    dparts_sb = sb.tile([P, NT, 5], F32, tag="dparts")
    dsum_sb = sb.tile([P, NT, 1], F32, tag="dsum")
    dtot_sb = sb.tile([P, NT], F32, tag="dtot")
    dinv_sb = sb.tile([P, NT], F32, tag="dinv")
    d_sb = sb.tile([P, NT], F32, tag="d")
    a_sb = sb.tile([P, C], F32R, tag="a")
    at_sb = sb.tile([P, CT, M], F32R, tag="at")
    nag_sb = sb.tile([P, C], F32R, tag="nag")

    # ---- input DMAs (x chunked so reduces/rounding can overlap) ----
    XCH = HW // 4
    for xc in range(4):
        for ci in range(CT):
            lo = xc * XCH
            nc.sync.dma_start(out=x_sb[:, ci, lo:lo + XCH],
                              in_=ftr[ci * P:(ci + 1) * P, lo:lo + XCH])
    nc.sync.dma_start(out=convw_sb, in_=convw[:, :])
    nc.sync.dma_start(out=convb_sb, in_=convb[:, :])
    nc.sync.dma_start(out=avgw_sb, in_=avgw[:, :])
    nc.sync.dma_start(out=maxw_sb, in_=maxw[:, :])
    for t in range(CT):
        nc.sync.dma_start(out=g_sb[:, t, :], in_=gcnw[t * P:(t + 1) * P, :])

    make_identity(nc, ident)
    nc.vector.tensor_copy(identr, ident)
    nc.vector.memset(q_sb, 0.0)
    nc.vector.memset(q_sb[:, P - 1:P], 1.0)
    nc.vector.memset(ones_sb, 1.0)
    nc.vector.memset(dparts_sb, 0.0)

    # fp32r rounding copies of DMA-landed matmul operands (chunked: overlap DMA;
    # on gpsimd to relieve DVE)
    for ci in range(CT):
        for xc in range(4):
            lo = xc * XCH
            nc.gpsimd.tensor_copy(xr_sb[:, ci, lo:lo + XCH], x_sb[:, ci, lo:lo + XCH])
    nc.vector.tensor_copy(gr_sb[:, :, :], g_sb[:, :, :])
    # gplus = gcn_w + I (as lhsT this computes gcn_w^T x + x in one pass)
    nc.vector.tensor_copy(gplus_sb[:, :, :], g_sb[:, :, :])
    for t in range(CT):
        blk = gplus_sb[:, t, t * P:(t + 1) * P]
        nc.vector.tensor_add(blk, blk, identr)

    # ---- weight transposes: w[M, C] -> wT[c-tile][P, M] ----
    for w_sb, wT in ((convw_sb, convwT_sb), (avgw_sb, avgwT_sb), (maxw_sb, maxwT_sb)):
        ps = mmp.tile([P, BANK], F32, tag="mm")
        for ci in range(CT):
            nc.tensor.transpose(ps[:, ci * P:(ci + 1) * P], w_sb[:, ci * P:(ci + 1) * P], ident)
        nc.vector.tensor_copy(wT[:, :, :], ps[:, :C])

    # ---- pooled stats (free-axis reduce is DVE-only); chunked to overlap DMA ----
    avgp_sb = sb.tile([P, CT, 4], F32, tag="avgp")
    mxp_sb = sb.tile([P, CT, 4], F32, tag="mxp")
    for ci in range(CT):
        for xc in range(4):
            lo = xc * XCH
            nc.vector.reduce_sum(out=avgp_sb[:, ci, xc:xc + 1],
                                 in_=x_sb[:, ci, lo:lo + XCH], axis=AX.X)
            nc.vector.reduce_max(out=mxp_sb[:, ci, xc:xc + 1],
                                 in_=x_sb[:, ci, lo:lo + XCH], axis=AX.X)
    for ci in range(CT):
        nc.vector.reduce_sum(out=avg_sb[:, ci, :], in_=avgp_sb[:, ci, :], axis=AX.X)
        nc.vector.reduce_max(out=mx_sb[:, ci, :], in_=mxp_sb[:, ci, :], axis=AX.X)

    # ---- k = relu(Wk @ x + b) ----
    for off, sz in _chunks(HW, BANK):
        kps = mmp.tile([P, BANK], F32, tag="mm")
        for ci in range(CT):
            nc.tensor.matmul(kps[:, :sz], lhsT=convwT_sb[:, ci, :],
                             rhs=xr_sb[:, ci, off:off + sz],
                             start=(ci == 0), stop=(ci == CT - 1))
        nc.scalar.activation(out=k_sb[:, off:off + sz], in_=kps[:, :sz],
                             func=AF.Relu, bias=convb_sb[:, :])

    # ---- channel attention cw ----
    aps = mmp.tile([P, BANK], F32, tag="mm")
    for ci in range(CT):
        nc.tensor.matmul(aps[:, 0:1], lhsT=avgwT_sb[:, ci, :], rhs=avg_sb[:, ci, :],
                         start=(ci == 0), stop=(ci == CT - 1))
    nc.scalar.activation(out=ra_sb, in_=aps[:, 0:1], func=AF.Relu, scale=1.0 / HW)
    mps = mmp.tile([P, BANK], F32, tag="mm")
    for ci in range(CT):
        nc.tensor.matmul(mps[:, 0:1], lhsT=maxwT_sb[:, ci, :], rhs=mx_sb[:, ci, :],
                         start=(ci == 0), stop=(ci == CT - 1))
    nc.scalar.activation(out=rm_sb, in_=mps[:, 0:1], func=AF.Relu)
    nc.vector.tensor_add(cwin_sb, ra_sb, rm_sb)
    nc.scalar.activation(out=cw_sb, in_=cwin_sb, func=AF.Sigmoid)

    # ---- kq = cw * k (chunked: early strips unblock sooner) ----
    for off, sz in _chunks(HW, 768):
        nc.vector.tensor_scalar_mul(kq_sb[:, off:off + sz], k_sb[:, off:off + sz],
                                    cw_sb[:, :])

    # ---- transposes of x and k (overlap with score phase) ----
    for j0 in range(0, NT, 2):
        tp = mmp.tile([P, BANK], F32, tag="mm")
        for dj in range(2):
            j = j0 + dj
            for ci in range(CT):
                nc.tensor.transpose(tp[:, dj * C + ci * P: dj * C + (ci + 1) * P],
                                    x_sb[:, ci, j * P:(j + 1) * P], ident)
        nc.vector.tensor_copy(xT_sb[:, j0:j0 + 2, :], tp[:, :])
    for j0 in range(0, NT, 4):
        nj = min(4, NT - j0)
        tp = mmp.tile([P, BANK], F32, tag="mm")
        for dj in range(nj):
            j = j0 + dj
            nc.tensor.transpose(tp[:, dj * P:(dj + 1) * P],
                                k_sb[:, j * P:(j + 1) * P].bitcast(F32), ident)
        nc.vector.tensor_copy(kT_sb[:, j0:j0 + nj, :], tp[:, :nj * P])

    # gx = (I+g)^T x chunks (d-independent): interleaved into the score loop
    # below so they fill PE gaps while ACT is the bottleneck.
    def _gx_chunk(ci, off, sz):
        gp = mmp.tile([P, BANK], F32, tag="mm")
        for t in range(CT):
            nc.tensor.matmul(gp[:, :sz], lhsT=gplus_sb[:, t, ci * P:(ci + 1) * P],
                             rhs=xr_sb[:, t, off:off + sz],
                             start=(t == 0), stop=(t == CT - 1))
        nc.vector.tensor_copy(yT_sb[:, ci, off:off + sz], gp[:, :sz])

    gx_todo = [(ci, off, sz) for ci in range(CT) for off, sz in _chunks(HW, BANK)]

    # ---- score phase ----
    if TRIANGLE:
        # Upper-triangle strips of S = sigmoid(kq^T k) (S is symmetric).
        # Row-sums: ACT sigmoid+accumulate.  Column-sums (lower-triangle
        # fill): PE ones-column matmuls into persistent PSUM G[strip, col].
        with tc.tile_pool(name="gps", bufs=1, space="PSUM") as gpool, \
             tc.tile_pool(name="sps", bufs=2, space="PSUM") as sps:
            G_ps = gpool.tile([P, HW], F32, tag="G")
            for i in range(NT):
                lhsT = kq_sb[:, i * P:(i + 1) * P]
                for cidx, (off, sz) in enumerate(_bank_chunks(i * P, HW)):
                    sp = sps.tile([P, BANK], F32, tag="s")
                    nc.tensor.matmul(sp[:, :sz], lhsT=lhsT,
                                     rhs=k_sb[:, off:off + sz], start=True, stop=True)
                    sig = scr.tile([P, BANK], BF16, tag="sig")
                    nc.scalar.activation(out=sig[:, :sz], in_=sp[:, :sz], func=AF.Sigmoid,
                                         accum_out=dparts_sb[:, i, cidx:cidx + 1])
                    # strict-upper column sums -> G[i, col]
                    lo = max(off, (i + 1) * P)
                    if lo < off + sz and i <= NT - 2:
                        b = off // BANK
                        last = (i == min(NT - 2, 4 * b + 2))
                        nc.tensor.matmul(
                            G_ps[:, lo:off + sz],
                            lhsT=q_sb[:, P - 1 - i:2 * P - 1 - i],
                            rhs=sig[:, lo - off:sz],
                            start=(i == 0), stop=last)
                        if last:
                            # bank b of G is final; spill it now (off the tail)
                            glo = max(P, b * BANK)
                            ghi = min(HW, (b + 1) * BANK)
                            nc.vector.tensor_copy(G_sb[:, glo:ghi], G_ps[:, glo:ghi])
                if i >= 7 and gx_todo:
                    _gx_chunk(*gx_todo.pop(0))
    else:
        # Full S, row-sums only via ACT sigmoid+accumulate.
        with tc.tile_pool(name="sps", bufs=2, space="PSUM") as sps:
            for i in range(NT):
                lhsT = kq_sb[:, i * P:(i + 1) * P]
                for h in range(2):
                    sp = sps.tile([P, 1152], F32, tag="s")
                    base = h * 1152
                    for off, sz in _chunks(1152, BANK):
                        nc.tensor.matmul(sp[:, off:off + sz], lhsT=lhsT,
                                         rhs=k_sb[:, base + off:base + off + sz],
                                         start=True, stop=True)
                    sig = scr.tile([P, 1152], BF16, tag="sig")
                    nc.scalar.activation(out=sig, in_=sp, func=AF.Sigmoid,
                                         accum_out=dparts_sb[:, i, h:h + 1])

    while gx_todo:
        _gx_chunk(*gx_todo.pop(0))

    # tail psum pool (score pools released)
    with tc.tile_pool(name="tailp", bufs=4, space="PSUM") as tailp:
        # ---- d = (rowsum + colsum)^-1/2 ----
        nc.vector.reduce_sum(out=dsum_sb[:, :, :], in_=dparts_sb[:, :, :], axis=AX.X)
        if TRIANGLE:
            dcol_ps = tailp.tile([P, 32], F32, tag="t")
            for j in range(1, NT):
                nc.tensor.matmul(dcol_ps[:, j:j + 1], lhsT=G_sb[:, j * P:(j + 1) * P],
                                 rhs=ones_sb[:, :], start=True, stop=True)
            nc.vector.tensor_copy(dtot_sb[:, 0:1], dsum_sb[:, 0, :])
            nc.vector.tensor_add(dtot_sb[:, 1:NT], dsum_sb[:, 1:NT, 0], dcol_ps[:, 1:NT])
        else:
            nc.vector.tensor_copy(dtot_sb[:, :], dsum_sb[:, :, 0])
        nc.vector.reciprocal(out=dinv_sb[:, :], in_=dtot_sb[:, :])
        nc.scalar.activation(out=d_sb[:, :], in_=dinv_sb[:, :], func=AF.Sqrt)

        # ---- kdT = d * kT ; kd = cw * transpose(kdT) ----
        for nt in range(NT):
            nc.gpsimd.tensor_scalar_mul(kdT_sb[:, nt, :], kT_sb[:, nt, :],
                                        d_sb[:, nt:nt + 1])
        for j0 in range(0, NT, 4):
            nj = min(4, NT - j0)
            tp = mmp.tile([P, BANK], F32, tag="mm")
            for dj in range(nj):
                j = j0 + dj
                nc.tensor.transpose(tp[:, dj * P:(dj + 1) * P],
                                    kdT_sb[:, j, :].bitcast(F32), ident)
            nc.vector.tensor_scalar_mul(kd_sb[:, j0 * P:(j0 + nj) * P],
                                        tp[:, :nj * P], cw_sb[:, :])

        # ---- A = (d k) @ x^T;  nAG = -(A @ gcn_w) ----
        a_ps = tailp.tile([P, BANK], F32, tag="t")
        for nt in range(NT):
            nc.tensor.matmul(a_ps[:, :C], lhsT=kdT_sb[:, nt, :], rhs=xT_sb[:, nt, :],
                             start=(nt == 0), stop=(nt == NT - 1))
        nc.vector.tensor_copy(a_sb[:, :], a_ps[:, :C])
        at_ps = tailp.tile([P, BANK], F32, tag="t")
        for ci in range(CT):
            nc.tensor.transpose(at_ps[:, ci * P:(ci + 1) * P],
                                a_sb[:, ci * P:(ci + 1) * P].bitcast(F32), ident)
        nc.vector.tensor_copy(at_sb[:, :, :], at_ps[:, :C])
        ag_ps = tailp.tile([P, BANK], F32, tag="t")
        for t in range(CT):
            nc.tensor.matmul(ag_ps[:, :C], lhsT=at_sb[:, t, :],
                             rhs=gr_sb[:, t, :], start=(t == 0), stop=(t == CT - 1))
        nc.vector.tensor_scalar_mul(nag_sb[:, :], ag_ps[:, :C], -1.0)

        # ---- out = gx - AG-contraction kd  (gx already in yT_sb) ----
        for ci in range(CT):
            for off, sz in _chunks(HW, BANK):
                yp = tailp.tile([P, BANK], F32, tag="t")
                nc.tensor.matmul(yp[:, :sz], lhsT=nag_sb[:, ci * P:(ci + 1) * P],
                                 rhs=kd_sb[:, off:off + sz], start=True, stop=True)
                dst = yT_sb[:, ci, off:off + sz]
                nc.vector.tensor_add(dst, dst, yp[:, :sz])
                nc.sync.dma_start(out=out[ci * P:(ci + 1) * P, off:off + sz],
                                  in_=dst)


_PROGRAM = None


def _get_program():
    global _PROGRAM
    if _PROGRAM is None:
        _PROGRAM = build_program()
    return _PROGRAM


def _in_maps(ftr, conv_k_w, conv_k_b, avg_fc_w, max_fc_w, gcn_w):
    wmaps = {
        "convw": np.ascontiguousarray(conv_k_w, dtype=np.float32),
        "convb": np.ascontiguousarray(np.asarray(conv_k_b, dtype=np.float32).reshape(M, 1)),
        "avgw": np.ascontiguousarray(avg_fc_w, dtype=np.float32),
        "maxw": np.ascontiguousarray(max_fc_w, dtype=np.float32),
        "gcnw": np.ascontiguousarray(gcn_w, dtype=np.float32),
    }
    return [
        {"ftr": np.ascontiguousarray(np.asarray(ftr[b], dtype=np.float32).reshape(C, HW)), **wmaps}
        for b in range(B)
    ]


def kernel(ftr, conv_k_w, conv_k_b, avg_fc_w, max_fc_w, gcn_w):
    nc = _get_program()
    in_maps = _in_maps(ftr, conv_k_w, conv_k_b, avg_fc_w, max_fc_w, gcn_w)
    res = run_bass_kernel_spmd(nc, in_maps, core_ids=list(range(N_CORES)))
    outs = [np.asarray(res.results[b]["out"]).reshape(C, H, W) for b in range(B)]
    return np.stack(outs, axis=0).astype(np.float32)

